# revision 3
# baseline (speedup 1.0000x reference)
"""CAM (channel attention) module kernel for Trainium2 (Bass/Tile).

Reference computation (per batch b):
    energy  = x_b @ x_b.T                      # [C, C], contraction over N
    att     = softmax(rowmax(energy) - energy) # row-wise over last axis
    out     = att @ x_b                        # [C, N]
    y_b     = gamma * out + x_b
Identity: softmax(rowmax(E) - E)[i,j] = exp(mn[i] - E[i,j]) / Z[i],
mn[i] = min_j E[i,j] (shift invariance; exact).

Sharding: data-parallel over B across 8 NeuronCores (B=32 -> 4 per core),
gamma replicated, full CxC attention per core.

v3 design (vs v1: PE transposes + GpSimd bulk cast):
  - X arrives via GpSimd SWDGE DMA with in-flight f32->f16 cast straight
    into X16 window tiles (no staging, no engine cast).
  - xT for matmul-1 produced by DMA XBAR transposes (dma_start_transpose,
    2-byte) -- zero PE transpose cycles.
  - E split in two PSUM halves (rows 0-1, rows 2-3) so softmax of the
    first half overlaps matmul-1 of the second, and E banks release
    early for the next batch.
  - tS (exp) in f16, tT via DMA transposes as well.
  - matmul-2 f16: stationary tT chunks, moving X16 windows; evac on DVE
    (x gamma/Z + residual from X16).
  - Software-pipelined emission: iter k emits in-DMA(k+1), xT(k+1),
    mm1+softmax(k), mm2(k-1).
PSUM: E_lo 2 banks + E_hi 2 banks + 4 accumulation banks.
"""

import contextlib

import numpy as np

P = 128

_CACHE = {}


DEFAULT_OPTS = dict(
    acc_bufs=4,     # PSUM banks for matmul-2 accumulation
    o_bufs=3,       # output window staging tiles [P, CO, 512] f32
    x16_bufs=2,     # per-window X16 ring depth
    xt_bufs=2,      # xt slab ring depth
    timing_io=False,  # x/y internal DRAM (no host transfer) -- timing runs
)


def _build(Bs, C, N, reps=1, **opts):
    import concourse.bass as bass  # noqa: F401
    import concourse.tile as tile
    import concourse.mybir as mybir
    from concourse import bacc
    from concourse.masks import make_identity

    o = dict(DEFAULT_OPTS)
    o.update(opts)

    F32 = mybir.dt.float32
    F16 = mybir.dt.float16
    AF = mybir.ActivationFunctionType
    ALU = mybir.AluOpType
    AX = mybir.AxisListType

    assert C == 4 * P and N % 1024 == 0
    CO = C // P          # 4 row/col chunks of 128
    KC = N // P          # 32 contraction chunks of 128
    NW = N // 512        # 8 n-windows of 512 (X16 window tiles, mm2 free dim)
    WP = NW // 2         # 4 window pairs (xT transpose granules of 1024)

    nc = bacc.Bacc(None, target_bir_lowering=False, debug=False)
    if o["timing_io"]:
        x_in = nc.dram_tensor("x_int", [Bs, C, N], F32)
        g_in = nc.dram_tensor("gamma", [1], F32, kind="ExternalInput")
        y_out = nc.dram_tensor("y_int", [Bs, C, N], F32)
        yy_out = nc.dram_tensor("yy", [1, 1], F32, kind="ExternalOutput")
    else:
        x_in = nc.dram_tensor("x", [Bs, C, N], F32, kind="ExternalInput")
        g_in = nc.dram_tensor("gamma", [1], F32, kind="ExternalInput")
        y_out = nc.dram_tensor("y", [Bs, C, N], F32, kind="ExternalOutput")
        yy_out = None

    with tile.TileContext(nc) as tc:
        with (
            tc.tile_pool(name="consts", bufs=1) as consts,
            tc.tile_pool(name="x16p", bufs=o["x16_bufs"]) as x16p,
            tc.tile_pool(name="xtp", bufs=o["xt_bufs"]) as xtp,
            tc.tile_pool(name="tsp", bufs=2) as tsp,
            tc.tile_pool(name="ttp", bufs=2) as ttp,
            tc.tile_pool(name="otp", bufs=o["o_bufs"]) as otp,
            tc.tile_pool(name="stgp", bufs=2) as stgp,
            tc.tile_pool(name="stats", bufs=2) as stats,
            tc.tile_pool(name="pelo", bufs=1, space="PSUM") as psum_lo,
            tc.tile_pool(name="pehi", bufs=1, space="PSUM") as psum_hi,
            tc.tile_pool(name="pacc", bufs=o["acc_bufs"], space="PSUM") as psum_acc,
        ):
            ident = consts.tile([P, P], F32)
            make_identity(nc, ident)
            g_sb = consts.tile([1, 1], F32)
            nc.sync.dma_start(g_sb[:, :], g_in[:].rearrange("(a b) -> a b", a=1))
            g_col = consts.tile([P, 1], F32)
            nc.gpsimd.partition_broadcast(g_col[:, :], g_sb[:1, :1])

            if o["timing_io"]:
                zt = otp.tile([P, CO, 512], F32, tag="ot", name="zt")
                nc.gpsimd.memset(zt[:, :, :], 0.0)
                for zb in range(Bs):
                    zx = x_in[zb].rearrange("(co p) n -> p co n", p=P)
                    for zw in range(NW):
                        nc.sync.dma_start(
                            zx[:, :, zw * 512:(zw + 1) * 512], zt[:, :, :]
                        )

            def emit_load2(b):
                """in-DMA (f32->f16 cast on SWDGE) + per-window xT DMA
                transposes ([128, 512] -> [128, 4, 128] blocks)."""
                x_b = x_in[b].rearrange("(co p) n -> p co n", p=P)
                X16w = [
                    x16p.tile([P, CO, 512], F16, tag=f"x16w{w}", name=f"X16w{w}")
                    for w in range(NW)
                ]
                xt = xtp.tile([P, KC, C], F16, tag="xt", name="xt")
                for w in range(NW):
                    ws = slice(w * 512, (w + 1) * 512)
                    nc.gpsimd.dma_start(X16w[w][:, :, :], x_b[:, :, ws])
                    for co in range(CO):
                        nc.sync.dma_start_transpose(
                            xt[:, w * 4:(w + 1) * 4, co * P:(co + 1) * P],
                            X16w[w][:, co, :],
                        )
                return X16w, xt

            def emit_mm1(b, st):
                """mm1 both pairs + mirrors + softmax + tT transposes."""
                xt = st["xt"]
                E_lo = psum_lo.tile([P, 2, C], F32, tag="Elo", name="E_lo")
                E_hi = psum_hi.tile([P, 2, C], F32, tag="Ehi", name="E_hi")
                mn = stats.tile([P, CO], F32, tag="mn")
                zs = stats.tile([P, CO], F32, tag="zs")
                rg = stats.tile([P, CO], F32, tag="rg")
                tS = tsp.tile([P, CO, C], F16, tag="tS")
                tT = ttp.tile([P, CO, C], F16, tag="tT")

                # ---- pair 01: rows 0 (cols 0:512) and 1 (cols 128:512)
                for kc in range(KC):
                    nc.tensor.matmul(
                        E_lo[:, 0, :], xt[:, kc, 0:P], xt[:, kc, :],
                        start=(kc == 0), stop=(kc == KC - 1),
                    )
                    nc.tensor.matmul(
                        E_lo[:, 1, P:], xt[:, kc, P:2 * P], xt[:, kc, P:],
                        start=(kc == 0), stop=(kc == KC - 1),
                    )
                # mirror E[1, 0:128] = E[0, 128:256]^T
                s01 = stgp.tile([P, P], F32, tag="s01")
                nc.scalar.copy(s01[:, :], E_lo[:, 0, P:2 * P])
                nc.tensor.matmul(
                    E_lo[:, 1, 0:P], s01[:, :], ident,
                    is_transpose=True, skip_group_check=True,
                )
                # stages for pair-23 mirrors (all reads of E_lo)
                s02 = stgp.tile([P, P], F32, tag="s02")
                s12 = stgp.tile([P, P], F32, tag="s12")
                s03 = stgp.tile([P, P], F32, tag="s03")
                s13 = stgp.tile([P, P], F32, tag="s13")
                nc.scalar.copy(s02[:, :], E_lo[:, 0, 2 * P:3 * P])
                nc.scalar.copy(s12[:, :], E_lo[:, 1, 2 * P:3 * P])
                nc.scalar.copy(s03[:, :], E_lo[:, 0, 3 * P:4 * P])
                nc.scalar.copy(s13[:, :], E_lo[:, 1, 3 * P:4 * P])
                # softmax rows 0,1 (frees E_lo afterwards)
                for ic in (0, 1):
                    nc.vector.tensor_reduce(
                        mn[:, ic:ic + 1], E_lo[:, ic, :], AX.X, ALU.min
                    )
                for ic in (0, 1):
                    nc.scalar.activation(
                        tS[:, ic, :], E_lo[:, ic, :], AF.Exp,
                        bias=mn[:, ic:ic + 1], scale=-1.0,
                        accum_out=zs[:, ic:ic + 1],
                    )

                # ---- pair 23: rows 2 (cols 256:512), 3 (cols 384:512)
                for kc in range(KC):
                    nc.tensor.matmul(
                        E_hi[:, 0, 2 * P:], xt[:, kc, 2 * P:3 * P],
                        xt[:, kc, 2 * P:],
                        start=(kc == 0), stop=(kc == KC - 1),
                    )
                    nc.tensor.matmul(
                        E_hi[:, 1, 3 * P:], xt[:, kc, 3 * P:4 * P],
                        xt[:, kc, 3 * P:],
                        start=(kc == 0), stop=(kc == KC - 1),
                    )
                # mirrors into E_hi
                nc.tensor.matmul(E_hi[:, 0, 0:P], s02[:, :], ident,
                                 is_transpose=True, skip_group_check=True)
                nc.tensor.matmul(E_hi[:, 0, P:2 * P], s12[:, :], ident,
                                 is_transpose=True, skip_group_check=True)
                nc.tensor.matmul(E_hi[:, 1, 0:P], s03[:, :], ident,
                                 is_transpose=True, skip_group_check=True)
                nc.tensor.matmul(E_hi[:, 1, P:2 * P], s13[:, :], ident,
                                 is_transpose=True, skip_group_check=True)
                s23 = stgp.tile([P, P], F32, tag="s23")
                nc.scalar.copy(s23[:, :], E_hi[:, 0, 3 * P:4 * P])
                nc.tensor.matmul(E_hi[:, 1, 2 * P:3 * P], s23[:, :], ident,
                                 is_transpose=True, skip_group_check=True)
                # softmax rows 2,3
                for ic in (2, 3):
                    nc.vector.tensor_reduce(
                        mn[:, ic:ic + 1], E_hi[:, ic - 2, :], AX.X, ALU.min
                    )
                for ic in (2, 3):
                    nc.scalar.activation(
                        tS[:, ic, :], E_hi[:, ic - 2, :], AF.Exp,
                        bias=mn[:, ic:ic + 1], scale=-1.0,
                        accum_out=zs[:, ic:ic + 1],
                    )
                nc.vector.reciprocal(rg[:, :], zs[:, :])
                nc.vector.tensor_scalar_mul(rg[:, :], rg[:, :], g_col[:, :1])
                # tT[j, jc, i] via DMA transposes of tS rows
                for ic in range(CO):
                    nc.sync.dma_start_transpose(
                        tT[:, :, ic * P:(ic + 1) * P], tS[:, ic, :]
                    )
                st["tT"] = tT
                st["rg"] = rg

            def emit_mm2(b, st):
                """mm2 + evac + out-DMA, per n-window."""
                X16w, tT, rg = st["X16w"], st["tT"], st["rg"]
                y_b = y_out[b].rearrange("(co p) n -> p co n", p=P)
                for w in range(NW):
                    ot = otp.tile([P, CO, 512], F32, tag="ot")
                    for ic in range(CO):
                        ps2 = psum_acc.tile([P, 512], F32, tag="acc")
                        for jc in range(CO):
                            nc.tensor.matmul(
                                ps2[:, :],
                                tT[:, jc, ic * P:(ic + 1) * P],
                                X16w[w][:, jc, :],
                                start=(jc == 0), stop=(jc == CO - 1),
                            )
                        nc.vector.scalar_tensor_tensor(
                            ot[:, ic, :], ps2[:, :], rg[:, ic:ic + 1],
                            X16w[w][:, ic, :],
                            op0=ALU.mult, op1=ALU.add,
                        )
                    nc.gpsimd.dma_start(
                        y_b[:, :, w * 512:(w + 1) * 512], ot[:, :, :]
                    )

            loop_ctx = (
                tc.For_i(0, reps, 1) if reps > 1 else contextlib.nullcontext()
            )
            with loop_ctx:
                state = {}
                for k in range(Bs + 1):
                    if k < Bs:
                        X16w, xt = emit_load2(k)
                        state[k] = {"X16w": X16w, "xt": xt}
                        emit_mm1(k, state[k])
                    if k >= 1:
                        emit_mm2(k - 1, state.pop(k - 1))

            if o["timing_io"]:
                ysb = stats.tile([1, 1], F32, tag="ysb")
                nc.sync.dma_start(
                    ysb[:1, :1], y_out[Bs - 1, C - 1:C, N - 1:N]
                )
                nc.sync.dma_start(yy_out[:1, :1], ysb[:1, :1])

    nc.compile()
    return nc


def get_nc(Bs=4, C=512, N=4096, reps=1, **opts):
    key = (Bs, C, N, reps, tuple(sorted(opts.items())))
    if key not in _CACHE:
        _CACHE[key] = _build(Bs, C, N, reps, **opts)
    return _CACHE[key]


def kernel(x, gamma):
    """Full inputs in, full output out. x [32, 512, 4096] f32, gamma [1] f32."""
    from concourse.bass_utils import run_bass_kernel_spmd

    x = np.ascontiguousarray(np.asarray(x, dtype=np.float32))
    gamma = np.ascontiguousarray(np.asarray(gamma, dtype=np.float32))
    B, C, N = x.shape
    n_cores = 8
    assert B % n_cores == 0
    Bs = B // n_cores

    nc = get_nc(Bs, C, N)
    in_maps = [
        {"x": x[i * Bs:(i + 1) * Bs], "gamma": gamma} for i in range(n_cores)
    ]
    res = run_bass_kernel_spmd(nc, in_maps, core_ids=list(range(n_cores)))
    return np.concatenate([r["y"] for r in res.results], axis=0)


# revision 4
# speedup vs baseline: 1.1545x; 1.1545x over previous
"""CAM (channel attention) module kernel for Trainium2 (Bass/Tile).

Reference computation (per batch b):
    energy  = x_b @ x_b.T                      # [C, C], contraction over N
    att     = softmax(rowmax(energy) - energy) # row-wise over last axis
    out     = att @ x_b                        # [C, N]
    y_b     = gamma * out + x_b
Identity: softmax(rowmax(E) - E)[i,j] = exp(mn[i] - E[i,j]) / Z[i],
mn[i] = min_j E[i,j] (shift invariance; exact).

Sharding: data-parallel over B across 8 NeuronCores (B=32 -> 4 per core),
gamma replicated, full CxC attention per core.

v3b design (measured-informed):
  - X arrives via GpSimd SWDGE DMA with in-flight f32->f16 cast straight
    into per-window X16 tiles (no staging, no engine cast; the v1 GpSimd
    bulk cast cost 220us/core and stalled the PE ~65us).
  - f16 (not bf16) for all 16-bit data: same PE speed, 8x the mantissa.
  - xT for matmul-1: PE transposes via *normal* matmul against an f16
    identity (1 cy/row; v1 used is_transpose on f32 data = 2 cy/row).
    One co-lane per window goes to the DMA XBAR (dma_start_transpose)
    to shave PE cycles; measured XBAR cost is ~1.25us per [128,512]
    serialized on the issuing engine, so only a slice of the work goes
    there (sync + scalar alternate).
  - tT via DMA XBAR transposes of tS (f16) -- no PE/PSUM/ScalarE cost.
  - matmul-2 f16: stationary tT chunks, moving X16 windows; evac on DVE
    (x gamma/Z + f16 residual from X16, ~2^-11 rel err).
  - Software-pipelined emission: iteration k emits mm1(k) then mm2(k-1);
    mm2(k-1) windows interleave the in-DMAs of batch k+1, so input
    windows land exactly one batch ahead of use and X16 buffers recycle
    per-window (bufs=2 per window tag).
PSUM: E 4 banks + 2 matmul-2 accumulation banks + 2 transpose banks.
"""

import contextlib

import numpy as np

P = 128

_CACHE = {}


DEFAULT_OPTS = dict(
    o_bufs=3,       # output window staging tiles [P, CO, 512] f32
    xbar_cos=(3,),  # co lanes whose xT transposes go to the DMA XBAR
    timing_io=False,  # x/y internal DRAM (no host transfer) -- timing runs
)


def _build(Bs, C, N, reps=1, **opts):
    import concourse.bass as bass  # noqa: F401
    import concourse.tile as tile
    import concourse.mybir as mybir
    from concourse import bacc
    from concourse.masks import make_identity

    o = dict(DEFAULT_OPTS)
    o.update(opts)

    F32 = mybir.dt.float32
    F16 = mybir.dt.float16
    AF = mybir.ActivationFunctionType
    ALU = mybir.AluOpType
    AX = mybir.AxisListType

    assert C == 4 * P and N % 512 == 0
    CO = C // P          # 4 row/col chunks of 128
    KC = N // P          # 32 contraction chunks of 128
    NW = N // 512        # 8 n-windows of 512

    xbar_cos = set(o["xbar_cos"])
    pe_cos = [co for co in range(CO) if co not in xbar_cos]

    nc = bacc.Bacc(None, target_bir_lowering=False, debug=False)
    if o["timing_io"]:
        x_in = nc.dram_tensor("x_int", [Bs, C, N], F32)
        g_in = nc.dram_tensor("gamma", [1], F32, kind="ExternalInput")
        y_out = nc.dram_tensor("y_int", [Bs, C, N], F32)
        yy_out = nc.dram_tensor("yy", [1, 1], F32, kind="ExternalOutput")
    else:
        x_in = nc.dram_tensor("x", [Bs, C, N], F32, kind="ExternalInput")
        g_in = nc.dram_tensor("gamma", [1], F32, kind="ExternalInput")
        y_out = nc.dram_tensor("y", [Bs, C, N], F32, kind="ExternalOutput")
        yy_out = None

    with tile.TileContext(nc) as tc:
        with (
            tc.tile_pool(name="consts", bufs=1) as consts,
            tc.tile_pool(name="x16p", bufs=2) as x16p,
            tc.tile_pool(name="xtp", bufs=2) as xtp,
            tc.tile_pool(name="tsp", bufs=2) as tsp,
            tc.tile_pool(name="ttp", bufs=2) as ttp,
            tc.tile_pool(name="otp", bufs=o["o_bufs"]) as otp,
            tc.tile_pool(name="stgp", bufs=2) as stgp,
            tc.tile_pool(name="stats", bufs=2) as stats,
            tc.tile_pool(name="pe", bufs=1, space="PSUM") as psum_e,
            tc.tile_pool(name="pacc", bufs=2, space="PSUM") as psum_acc,
            tc.tile_pool(name="psx", bufs=1, space="PSUM") as psum_xt,
        ):
            ident16 = consts.tile([P, P], F16)
            make_identity(nc, ident16)
            ident32 = consts.tile([P, P], F32)
            make_identity(nc, ident32)
            g_sb = consts.tile([1, 1], F32)
            nc.sync.dma_start(g_sb[:, :], g_in[:].rearrange("(a b) -> a b", a=1))
            g_col = consts.tile([P, 1], F32)
            nc.gpsimd.partition_broadcast(g_col[:, :], g_sb[:1, :1])

            if o["timing_io"]:
                zt = otp.tile([P, CO, 512], F32, tag="ot", name="zt")
                nc.gpsimd.memset(zt[:, :, :], 0.0)
                for zb in range(Bs):
                    zx = x_in[zb].rearrange("(co p) n -> p co n", p=P)
                    for zw in range(NW):
                        nc.sync.dma_start(
                            zx[:, :, zw * 512:(zw + 1) * 512], zt[:, :, :]
                        )

            st = {}  # per-batch live tiles

            def make_x16(b):
                st[b] = {
                    "X16w": [
                        x16p.tile([P, CO, 512], F16, tag=f"x16w{w}",
                                  name=f"X16w{w}")
                        for w in range(NW)
                    ]
                }

            def in_dma(b, w):
                x_b = x_in[b].rearrange("(co p) n -> p co n", p=P)
                nc.gpsimd.dma_start(
                    st[b]["X16w"][w][:, :, :],
                    x_b[:, :, w * 512:(w + 1) * 512],
                )

            def emit_mm1(b):
                X16w = st[b]["X16w"]
                xt = xtp.tile([P, KC, C], F16, tag="xt", name="xt")
                E = psum_e.tile([P, CO, C], F32, tag="E", name="E")
                mn = stats.tile([P, CO], F32, tag="mn")
                zs = stats.tile([P, CO], F32, tag="zs")
                rg = stats.tile([P, CO], F32, tag="rg")
                tS = tsp.tile([P, CO, C], F16, tag="tS")
                tT = ttp.tile([P, CO, C], F16, tag="tT")

                # XBAR transposes for the offloaded co lanes (run ahead,
                # serialized per issuing engine; sync/scalar alternate)
                for w in range(NW):
                    for i, co in enumerate(sorted(xbar_cos)):
                        eng = nc.sync if (w + i) % 2 == 0 else nc.scalar
                        eng.dma_start_transpose(
                            xt[:, w * 4:(w + 1) * 4, co * P:(co + 1) * P],
                            X16w[w][:, co, :],
                        )

                # PE transposes (normal matmul vs f16 identity) for window
                # w+1 interleave with matmul-1 chunks of window w
                def t_pe(w):
                    for i, co in enumerate(pe_cos):
                        ps = psum_xt.tile(
                            [P, 4, P], F32, tag=f"psx{(w * len(pe_cos) + i) % 2}",
                            bufs=1, name="ps_x",
                        )
                        for j in range(4):
                            nc.tensor.matmul(
                                ps[:, j, :],
                                X16w[w][:, co, j * P:(j + 1) * P],
                                ident16,
                            )
                        nc.scalar.copy(
                            xt[:, w * 4:(w + 1) * 4, co * P:(co + 1) * P],
                            ps[:, :, :],
                        )

                def mm1_chunks(w):
                    for kc in range(w * 4, (w + 1) * 4):
                        for ic in range(CO):
                            nc.tensor.matmul(
                                E[:, ic, ic * P:],
                                xt[:, kc, ic * P:(ic + 1) * P],
                                xt[:, kc, ic * P:],
                                start=(kc == 0),
                                stop=(kc == KC - 1),
                            )

                t_pe(0)
                for w in range(NW):
                    if w + 1 < NW:
                        t_pe(w + 1)
                    mm1_chunks(w)

                # mirror E[jc, ic] = E[ic, jc].T for ic < jc
                for jc in range(1, CO):
                    for ic in range(jc):
                        stg = stgp.tile([P, P], F32, tag="stg")
                        nc.scalar.copy(
                            stg[:, :], E[:, ic, jc * P:(jc + 1) * P]
                        )
                        nc.tensor.matmul(
                            E[:, jc, ic * P:(ic + 1) * P],
                            stg[:, :],
                            ident32,
                            is_transpose=True,
                            skip_group_check=True,
                        )

                # softmax: tS = exp(mn - E) in f16, Z row-sum fused (f32)
                for ic in range(CO):
                    nc.vector.tensor_reduce(
                        mn[:, ic:ic + 1], E[:, ic, :], AX.X, ALU.min
                    )
                for ic in range(CO):
                    nc.scalar.activation(
                        tS[:, ic, :], E[:, ic, :], AF.Exp,
                        bias=mn[:, ic:ic + 1], scale=-1.0,
                        accum_out=zs[:, ic:ic + 1],
                    )
                nc.vector.reciprocal(rg[:, :], zs[:, :])
                nc.vector.tensor_scalar_mul(rg[:, :], rg[:, :], g_col[:, :1])

                # tT[j, jc, i] via XBAR transposes of tS rows
                for ic in range(CO):
                    eng = nc.sync if ic % 2 == 0 else nc.scalar
                    eng.dma_start_transpose(
                        tT[:, :, ic * P:(ic + 1) * P], tS[:, ic, :]
                    )
                st[b]["tT"] = tT
                st[b]["rg"] = rg

            def emit_mm2(b, prefetch_b):
                """mm2 + evac + out-DMA per n-window; interleaves the
                in-DMAs of batch `prefetch_b` (X16 recycles per window)."""
                X16w, tT, rg = st[b]["X16w"], st[b]["tT"], st[b]["rg"]
                y_b = y_out[b].rearrange("(co p) n -> p co n", p=P)
                if prefetch_b is not None:
                    make_x16(prefetch_b)
                for w in range(NW):
                    ot = otp.tile([P, CO, 512], F32, tag="ot")
                    for ic in range(CO):
                        ps2 = psum_acc.tile([P, 512], F32, tag="acc")
                        for jc in range(CO):
                            nc.tensor.matmul(
                                ps2[:, :],
                                tT[:, jc, ic * P:(ic + 1) * P],
                                X16w[w][:, jc, :],
                                start=(jc == 0), stop=(jc == CO - 1),
                            )
                        nc.vector.scalar_tensor_tensor(
                            ot[:, ic, :], ps2[:, :], rg[:, ic:ic + 1],
                            X16w[w][:, ic, :],
                            op0=ALU.mult, op1=ALU.add,
                        )
                    nc.gpsimd.dma_start(
                        y_b[:, :, w * 512:(w + 1) * 512], ot[:, :, :]
                    )
                    if prefetch_b is not None:
                        in_dma(prefetch_b, w)
                del st[b]

            loop_ctx = (
                tc.For_i(0, reps, 1) if reps > 1 else contextlib.nullcontext()
            )
            with loop_ctx:
                for k in range(Bs + 1):
                    if k == 0:
                        make_x16(0)
                        for w in range(NW):
                            in_dma(0, w)
                        if Bs > 1:
                            make_x16(1)
                            for w in range(NW):
                                in_dma(1, w)
                    if k < Bs:
                        emit_mm1(k)
                    if k >= 1:
                        nb = k + 1 if k + 1 < Bs else None
                        emit_mm2(k - 1, nb)

            if o["timing_io"]:
                ysb = stats.tile([1, 1], F32, tag="ysb")
                nc.sync.dma_start(
                    ysb[:1, :1], y_out[Bs - 1, C - 1:C, N - 1:N]
                )
                nc.sync.dma_start(yy_out[:1, :1], ysb[:1, :1])

    nc.compile()
    return nc


def get_nc(Bs=4, C=512, N=4096, reps=1, **opts):
    key = (Bs, C, N, reps, tuple(sorted(opts.items())))
    if key not in _CACHE:
        _CACHE[key] = _build(Bs, C, N, reps, **opts)
    return _CACHE[key]


def kernel(x, gamma):
    """Full inputs in, full output out. x [32, 512, 4096] f32, gamma [1] f32."""
    from concourse.bass_utils import run_bass_kernel_spmd

    x = np.ascontiguousarray(np.asarray(x, dtype=np.float32))
    gamma = np.ascontiguousarray(np.asarray(gamma, dtype=np.float32))
    B, C, N = x.shape
    n_cores = 8
    assert B % n_cores == 0
    Bs = B // n_cores

    nc = get_nc(Bs, C, N)
    in_maps = [
        {"x": x[i * Bs:(i + 1) * Bs], "gamma": gamma} for i in range(n_cores)
    ]
    res = run_bass_kernel_spmd(nc, in_maps, core_ids=list(range(n_cores)))
    return np.concatenate([r["y"] for r in res.results], axis=0)


# revision 7
# speedup vs baseline: 1.7799x; 1.5417x over previous
"""CAM (channel attention) module kernel for Trainium2 (Bass/Tile).

Reference computation (per batch b):
    energy  = x_b @ x_b.T                      # [C, C], contraction over N
    att     = softmax(rowmax(energy) - energy) # row-wise over last axis
    out     = att @ x_b                        # [C, N]
    y_b     = gamma * out + x_b
Identity: softmax(rowmax(E) - E)[i,j] = exp(mn[i] - E[i,j]) / Z[i],
mn[i] = min_j E[i,j] (shift invariance; exact).

Sharding: data-parallel over B across 8 NeuronCores (B=32 -> 4 per core),
gamma replicated, full CxC attention per core.

v3b design (measured-informed):
  - X arrives via GpSimd SWDGE DMA with in-flight f32->f16 cast straight
    into per-window X16 tiles (no staging, no engine cast; the v1 GpSimd
    bulk cast cost 220us/core and stalled the PE ~65us).
  - f16 (not bf16) for all 16-bit data: same PE speed, 8x the mantissa.
  - xT for matmul-1: PE transposes via *normal* matmul against an f16
    identity (1 cy/row; v1 used is_transpose on f32 data = 2 cy/row).
    One co-lane per window goes to the DMA XBAR (dma_start_transpose)
    to shave PE cycles; measured XBAR cost is ~1.25us per [128,512]
    serialized on the issuing engine, so only a slice of the work goes
    there (sync + scalar alternate).
  - tT via DMA XBAR transposes of tS (f16) -- no PE/PSUM/ScalarE cost.
  - matmul-2 f16: stationary tT chunks, moving X16 windows; evac on DVE
    (x gamma/Z + f16 residual from X16, ~2^-11 rel err).
  - Software-pipelined emission: iteration k emits mm1(k) then mm2(k-1);
    mm2(k-1) windows interleave the in-DMAs of batch k+1, so input
    windows land exactly one batch ahead of use and X16 buffers recycle
    per-window (bufs=2 per window tag).
PSUM: E 4 banks + 2 matmul-2 accumulation banks + 2 transpose banks.
"""

import contextlib

import numpy as np

P = 128

_CACHE = {}


DEFAULT_OPTS = dict(
    o_bufs=3,       # output window staging tiles [P, CO, 512] f32
    xbar_cos=(),    # co lanes whose xT transposes go to the DMA XBAR
    timing_io=False,  # x/y internal DRAM (no host transfer) -- timing runs
)


def _build(Bs, C, N, reps=1, **opts):
    import concourse.bass as bass  # noqa: F401
    import concourse.tile as tile
    import concourse.mybir as mybir
    from concourse import bacc
    from concourse.masks import make_identity

    o = dict(DEFAULT_OPTS)
    o.update(opts)

    F32 = mybir.dt.float32
    F16 = mybir.dt.float16
    AF = mybir.ActivationFunctionType
    ALU = mybir.AluOpType
    AX = mybir.AxisListType

    assert C == 4 * P and N % 512 == 0
    CO = C // P          # 4 row/col chunks of 128
    KC = N // P          # 32 contraction chunks of 128
    NW = N // 512        # 8 n-windows of 512

    xbar_cos = set(o["xbar_cos"])
    pe_cos = [co for co in range(CO) if co not in xbar_cos]

    nc = bacc.Bacc(None, target_bir_lowering=False, debug=False)
    if o["timing_io"]:
        x_in = nc.dram_tensor("x_int", [Bs, C, N], F32)
        g_in = nc.dram_tensor("gamma", [1], F32, kind="ExternalInput")
        y_out = nc.dram_tensor("y_int", [Bs, C, N], F32)
        yy_out = nc.dram_tensor("yy", [1, 1], F32, kind="ExternalOutput")
    else:
        x_in = nc.dram_tensor("x", [Bs, C, N], F32, kind="ExternalInput")
        g_in = nc.dram_tensor("gamma", [1], F32, kind="ExternalInput")
        y_out = nc.dram_tensor("y", [Bs, C, N], F32, kind="ExternalOutput")
        yy_out = None

    with tile.TileContext(nc) as tc:
        with (
            tc.tile_pool(name="consts", bufs=1) as consts,
            tc.tile_pool(name="x16p", bufs=2) as x16p,
            tc.tile_pool(name="xtp", bufs=2) as xtp,
            tc.tile_pool(name="tsp", bufs=2) as tsp,
            tc.tile_pool(name="ttp", bufs=2) as ttp,
            tc.tile_pool(name="otp", bufs=o["o_bufs"]) as otp,
            tc.tile_pool(name="stgp", bufs=2) as stgp,
            tc.tile_pool(name="stats", bufs=2) as stats,
            tc.tile_pool(name="pe", bufs=1, space="PSUM") as psum_e,
            tc.tile_pool(name="pacc", bufs=2, space="PSUM") as psum_acc,
            tc.tile_pool(name="psx", bufs=1, space="PSUM") as psum_xt,
        ):
            ident16 = consts.tile([P, P], F16)
            make_identity(nc, ident16)
            ident32 = consts.tile([P, P], F32)
            make_identity(nc, ident32)
            g_sb = consts.tile([1, 1], F32)
            nc.sync.dma_start(g_sb[:, :], g_in[:].rearrange("(a b) -> a b", a=1))
            g_col = consts.tile([P, 1], F32)
            nc.gpsimd.partition_broadcast(g_col[:, :], g_sb[:1, :1])

            if o["timing_io"]:
                zt = otp.tile([P, CO, 512], F32, tag="ot", name="zt")
                nc.gpsimd.memset(zt[:, :, :], 0.0)
                for zb in range(Bs):
                    zx = x_in[zb].rearrange("(co p) n -> p co n", p=P)
                    for zw in range(NW):
                        nc.sync.dma_start(
                            zx[:, :, zw * 512:(zw + 1) * 512], zt[:, :, :]
                        )

            st = {}  # per-batch live tiles

            def make_x16(b):
                st[b] = {
                    "X16w": [
                        x16p.tile([P, CO, 512], F16, tag=f"x16w{w}",
                                  name=f"X16w{w}")
                        for w in range(NW)
                    ]
                }

            def in_dma(b, w):
                x_b = x_in[b].rearrange("(co p) n -> p co n", p=P)
                nc.gpsimd.dma_start(
                    st[b]["X16w"][w][:, :, :],
                    x_b[:, :, w * 512:(w + 1) * 512],
                )

            def emit_mm1(b):
                X16w = st[b]["X16w"]
                xt = xtp.tile([P, KC, C], F16, tag="xt", name="xt")
                E = psum_e.tile([P, CO, C], F32, tag="E", name="E")
                mn = stats.tile([P, CO], F32, tag="mn")
                zs = stats.tile([P, CO], F32, tag="zs")
                rg = stats.tile([P, CO], F32, tag="rg")
                tS = tsp.tile([P, CO, C], F16, tag="tS")
                tT = ttp.tile([P, CO, C], F16, tag="tT")

                # XBAR transposes for the offloaded co lanes (run ahead,
                # serialized per issuing engine; sync/scalar alternate)
                for w in range(NW):
                    for i, co in enumerate(sorted(xbar_cos)):
                        eng = nc.sync if (w + i) % 2 == 0 else nc.scalar
                        eng.dma_start_transpose(
                            xt[:, w * 4:(w + 1) * 4, co * P:(co + 1) * P],
                            X16w[w][:, co, :],
                        )

                # PE transposes (normal matmul vs f16 identity) for window
                # w+1 interleave with matmul-1 chunks of window w
                def t_pe(w):
                    for i, co in enumerate(pe_cos):
                        ps = psum_xt.tile(
                            [P, 4, P], F32, tag=f"psx{(w * len(pe_cos) + i) % 2}",
                            bufs=1, name="ps_x",
                        )
                        for j in range(4):
                            nc.tensor.matmul(
                                ps[:, j, :],
                                X16w[w][:, co, j * P:(j + 1) * P],
                                ident16,
                            )
                        nc.scalar.copy(
                            xt[:, w * 4:(w + 1) * 4, co * P:(co + 1) * P],
                            ps[:, :, :],
                        )

                def mm1_chunks(w):
                    for kc in range(w * 4, (w + 1) * 4):
                        for ic in range(CO):
                            nc.tensor.matmul(
                                E[:, ic, ic * P:],
                                xt[:, kc, ic * P:(ic + 1) * P],
                                xt[:, kc, ic * P:],
                                start=(kc == 0),
                                stop=(kc == KC - 1),
                            )

                t_pe(0)
                for w in range(NW):
                    if w + 1 < NW:
                        t_pe(w + 1)
                    mm1_chunks(w)

                # mirror E[jc, ic] = E[ic, jc].T for ic < jc
                for jc in range(1, CO):
                    for ic in range(jc):
                        stg = stgp.tile([P, P], F32, tag="stg")
                        nc.scalar.copy(
                            stg[:, :], E[:, ic, jc * P:(jc + 1) * P]
                        )
                        nc.tensor.matmul(
                            E[:, jc, ic * P:(ic + 1) * P],
                            stg[:, :],
                            ident32,
                            is_transpose=True,
                            skip_group_check=True,
                        )

                # softmax: tS = exp(mn - E) in f16, Z row-sum fused (f32)
                for ic in range(CO):
                    nc.vector.tensor_reduce(
                        mn[:, ic:ic + 1], E[:, ic, :], AX.X, ALU.min
                    )
                for ic in range(CO):
                    nc.scalar.activation(
                        tS[:, ic, :], E[:, ic, :], AF.Exp,
                        bias=mn[:, ic:ic + 1], scale=-1.0,
                        accum_out=zs[:, ic:ic + 1],
                    )
                nc.vector.reciprocal(rg[:, :], zs[:, :])
                nc.vector.tensor_scalar_mul(rg[:, :], rg[:, :], g_col[:, :1])

                # tT[j, jc, i] via XBAR transposes of tS rows (sync engine;
                # has a full mm2 window of slack before mm2(b) needs tT)
                for ic in range(CO):
                    nc.sync.dma_start_transpose(
                        tT[:, :, ic * P:(ic + 1) * P], tS[:, ic, :]
                    )
                st[b]["tT"] = tT
                st[b]["rg"] = rg

            def emit_mm2(b, prefetch_b):
                """mm2 + evac + out-DMA per n-window; interleaves the
                in-DMAs of batch `prefetch_b` (X16 recycles per window)."""
                X16w, tT, rg = st[b]["X16w"], st[b]["tT"], st[b]["rg"]
                y_b = y_out[b].rearrange("(co p) n -> p co n", p=P)
                if prefetch_b is not None:
                    make_x16(prefetch_b)
                for w in range(NW):
                    ot = otp.tile([P, CO, 512], F32, tag="ot")
                    for ic in range(CO):
                        ps2 = psum_acc.tile([P, 512], F32, tag="acc")
                        for jc in range(CO):
                            nc.tensor.matmul(
                                ps2[:, :],
                                tT[:, jc, ic * P:(ic + 1) * P],
                                X16w[w][:, jc, :],
                                start=(jc == 0), stop=(jc == CO - 1),
                            )
                        nc.vector.scalar_tensor_tensor(
                            ot[:, ic, :], ps2[:, :], rg[:, ic:ic + 1],
                            X16w[w][:, ic, :],
                            op0=ALU.mult, op1=ALU.add,
                        )
                    # out on sync HWDGE: SWDGE descriptors cost ~124ns/2KB
                    # vs HWDGE 17ns and would back up the DMA rings
                    nc.sync.dma_start(
                        y_b[:, :, w * 512:(w + 1) * 512], ot[:, :, :]
                    )
                    if prefetch_b is not None:
                        in_dma(prefetch_b, w)
                del st[b]

            loop_ctx = (
                tc.For_i(0, reps, 1) if reps > 1 else contextlib.nullcontext()
            )
            with loop_ctx:
                for k in range(Bs + 1):
                    if k == 0:
                        make_x16(0)
                        for w in range(NW):
                            in_dma(0, w)
                        if Bs > 1:
                            make_x16(1)
                            for w in range(NW):
                                in_dma(1, w)
                    if k < Bs:
                        emit_mm1(k)
                    if k >= 1:
                        nb = k + 1 if k + 1 < Bs else None
                        emit_mm2(k - 1, nb)

            if o["timing_io"]:
                ysb = stats.tile([1, 1], F32, tag="ysb")
                nc.sync.dma_start(
                    ysb[:1, :1], y_out[Bs - 1, C - 1:C, N - 1:N]
                )
                nc.sync.dma_start(yy_out[:1, :1], ysb[:1, :1])

    nc.compile()
    return nc


def get_nc(Bs=4, C=512, N=4096, reps=1, **opts):
    key = (Bs, C, N, reps, tuple(sorted(opts.items())))
    if key not in _CACHE:
        _CACHE[key] = _build(Bs, C, N, reps, **opts)
    return _CACHE[key]


def kernel(x, gamma):
    """Full inputs in, full output out. x [32, 512, 4096] f32, gamma [1] f32."""
    from concourse.bass_utils import run_bass_kernel_spmd

    x = np.ascontiguousarray(np.asarray(x, dtype=np.float32))
    gamma = np.ascontiguousarray(np.asarray(gamma, dtype=np.float32))
    B, C, N = x.shape
    n_cores = 8
    assert B % n_cores == 0
    Bs = B // n_cores

    nc = get_nc(Bs, C, N)
    in_maps = [
        {"x": x[i * Bs:(i + 1) * Bs], "gamma": gamma} for i in range(n_cores)
    ]
    res = run_bass_kernel_spmd(nc, in_maps, core_ids=list(range(n_cores)))
    return np.concatenate([r["y"] for r in res.results], axis=0)


# revision 9
# speedup vs baseline: 1.8655x; 1.0481x over previous
"""CAM (channel attention) module kernel for Trainium2 (Bass/Tile).

Reference computation (per batch b):
    energy  = x_b @ x_b.T                      # [C, C], contraction over N
    att     = softmax(rowmax(energy) - energy) # row-wise over last axis
    out     = att @ x_b                        # [C, N]
    y_b     = gamma * out + x_b
Identity: softmax(rowmax(E) - E)[i,j] = exp(mn[i] - E[i,j]) / Z[i],
mn[i] = min_j E[i,j] (shift invariance; exact).

Sharding: data-parallel over B across 8 NeuronCores (B=32 -> 4 per core),
gamma replicated, full CxC attention per core.

v3b design (measured-informed):
  - X arrives via GpSimd SWDGE DMA with in-flight f32->f16 cast straight
    into per-window X16 tiles (no staging, no engine cast; the v1 GpSimd
    bulk cast cost 220us/core and stalled the PE ~65us).
  - f16 (not bf16) for all 16-bit data: same PE speed, 8x the mantissa.
  - xT for matmul-1: PE transposes via *normal* matmul against an f16
    identity (1 cy/row; v1 used is_transpose on f32 data = 2 cy/row).
    One co-lane per window goes to the DMA XBAR (dma_start_transpose)
    to shave PE cycles; measured XBAR cost is ~1.25us per [128,512]
    serialized on the issuing engine, so only a slice of the work goes
    there (sync + scalar alternate).
  - tT via DMA XBAR transposes of tS (f16) -- no PE/PSUM/ScalarE cost.
  - matmul-2 f16: stationary tT chunks, moving X16 windows; evac on DVE
    (x gamma/Z + f16 residual from X16, ~2^-11 rel err).
  - Software-pipelined emission: iteration k emits mm1(k) then mm2(k-1);
    mm2(k-1) windows interleave the in-DMAs of batch k+1, so input
    windows land exactly one batch ahead of use and X16 buffers recycle
    per-window (bufs=2 per window tag).
PSUM: E 4 banks + 2 matmul-2 accumulation banks + 2 transpose banks.
"""

import contextlib

import numpy as np

P = 128

_CACHE = {}


DEFAULT_OPTS = dict(
    o_bufs=4,       # output window staging tiles [P, CO, 512] f32
    xbar_cos=(),    # co lanes whose xT transposes go to the DMA XBAR
    timing_io=False,  # x/y internal DRAM (no host transfer) -- timing runs
)


def _build(Bs, C, N, reps=1, **opts):
    import concourse.bass as bass  # noqa: F401
    import concourse.tile as tile
    import concourse.mybir as mybir
    from concourse import bacc
    from concourse.masks import make_identity

    o = dict(DEFAULT_OPTS)
    o.update(opts)

    F32 = mybir.dt.float32
    F16 = mybir.dt.float16
    AF = mybir.ActivationFunctionType
    ALU = mybir.AluOpType
    AX = mybir.AxisListType

    assert C == 4 * P and N % 512 == 0
    CO = C // P          # 4 row/col chunks of 128
    KC = N // P          # 32 contraction chunks of 128
    NW = N // 512        # 8 n-windows of 512

    xbar_cos = set(o["xbar_cos"])
    pe_cos = [co for co in range(CO) if co not in xbar_cos]

    nc = bacc.Bacc(None, target_bir_lowering=False, debug=False)
    if o["timing_io"]:
        x_in = nc.dram_tensor("x_int", [Bs, C, N], F32)
        g_in = nc.dram_tensor("gamma", [1], F32, kind="ExternalInput")
        y_out = nc.dram_tensor("y_int", [Bs, C, N], F32)
        yy_out = nc.dram_tensor("yy", [1, 1], F32, kind="ExternalOutput")
    else:
        x_in = nc.dram_tensor("x", [Bs, C, N], F32, kind="ExternalInput")
        g_in = nc.dram_tensor("gamma", [1], F32, kind="ExternalInput")
        y_out = nc.dram_tensor("y", [Bs, C, N], F32, kind="ExternalOutput")
        yy_out = None

    with tile.TileContext(nc) as tc:
        with (
            tc.tile_pool(name="consts", bufs=1) as consts,
            tc.tile_pool(name="x16p", bufs=2) as x16p,
            tc.tile_pool(name="xtp", bufs=2) as xtp,
            tc.tile_pool(name="tsp", bufs=2) as tsp,
            tc.tile_pool(name="ttp", bufs=2) as ttp,
            tc.tile_pool(name="otp", bufs=o["o_bufs"]) as otp,
            tc.tile_pool(name="stgp", bufs=2) as stgp,
            tc.tile_pool(name="stats", bufs=2) as stats,
            tc.tile_pool(name="pe", bufs=1, space="PSUM") as psum_e,
            tc.tile_pool(name="pacc", bufs=2, space="PSUM") as psum_acc,
            tc.tile_pool(name="psx", bufs=1, space="PSUM") as psum_xt,
        ):
            ident16 = consts.tile([P, P], F16)
            make_identity(nc, ident16)
            ident32 = consts.tile([P, P], F32)
            make_identity(nc, ident32)
            g_sb = consts.tile([1, 1], F32)
            nc.sync.dma_start(g_sb[:, :], g_in[:].rearrange("(a b) -> a b", a=1))
            g_col = consts.tile([P, 1], F32)
            nc.gpsimd.partition_broadcast(g_col[:, :], g_sb[:1, :1])

            if o["timing_io"]:
                zt = otp.tile([P, CO, 512], F32, tag="ot", name="zt")
                nc.gpsimd.memset(zt[:, :, :], 0.0)
                for zb in range(Bs):
                    zx = x_in[zb].rearrange("(co p) n -> p co n", p=P)
                    for zw in range(NW):
                        nc.sync.dma_start(
                            zx[:, :, zw * 512:(zw + 1) * 512], zt[:, :, :]
                        )

            st = {}  # per-batch live tiles

            def make_x16(b):
                st[b] = {
                    "X16w": [
                        x16p.tile([P, CO, 512], F16, tag=f"x16w{w}",
                                  name=f"X16w{w}")
                        for w in range(NW)
                    ]
                }

            def in_dma(b, w):
                x_b = x_in[b].rearrange("(co p) n -> p co n", p=P)
                nc.gpsimd.dma_start(
                    st[b]["X16w"][w][:, :, :],
                    x_b[:, :, w * 512:(w + 1) * 512],
                )

            def emit_mm1(b):
                X16w = st[b]["X16w"]
                xt = xtp.tile([P, KC, C], F16, tag="xt", name="xt")
                E = psum_e.tile([P, CO, C], F32, tag="E", name="E")
                mn = stats.tile([P, CO], F32, tag="mn")
                zs = stats.tile([P, CO], F32, tag="zs")
                rg = stats.tile([P, CO], F32, tag="rg")
                tS = tsp.tile([P, CO, C], F16, tag="tS")
                tT = ttp.tile([P, CO, C], F16, tag="tT")

                # XBAR transposes for the offloaded co lanes (run ahead,
                # serialized per issuing engine; sync/scalar alternate)
                for w in range(NW):
                    for i, co in enumerate(sorted(xbar_cos)):
                        eng = nc.sync if (w + i) % 2 == 0 else nc.scalar
                        eng.dma_start_transpose(
                            xt[:, w * 4:(w + 1) * 4, co * P:(co + 1) * P],
                            X16w[w][:, co, :],
                        )

                # PE transposes (normal matmul vs f16 identity) for window
                # w+1 interleave with matmul-1 chunks of window w
                def t_pe(w):
                    for i, co in enumerate(pe_cos):
                        ps = psum_xt.tile(
                            [P, 4, P], F32, tag=f"psx{(w * len(pe_cos) + i) % 2}",
                            bufs=1, name="ps_x",
                        )
                        for j in range(4):
                            nc.tensor.matmul(
                                ps[:, j, :],
                                X16w[w][:, co, j * P:(j + 1) * P],
                                ident16,
                            )
                        nc.scalar.copy(
                            xt[:, w * 4:(w + 1) * 4, co * P:(co + 1) * P],
                            ps[:, :, :],
                        )

                def mm1_chunks(w):
                    for kc in range(w * 4, (w + 1) * 4):
                        for ic in range(CO):
                            nc.tensor.matmul(
                                E[:, ic, ic * P:],
                                xt[:, kc, ic * P:(ic + 1) * P],
                                xt[:, kc, ic * P:],
                                start=(kc == 0),
                                stop=(kc == KC - 1),
                            )

                t_pe(0)
                for w in range(NW):
                    if w + 1 < NW:
                        t_pe(w + 1)
                    mm1_chunks(w)

                # mirror E[jc, ic] = E[ic, jc].T for ic < jc
                for jc in range(1, CO):
                    for ic in range(jc):
                        stg = stgp.tile([P, P], F32, tag="stg")
                        nc.scalar.copy(
                            stg[:, :], E[:, ic, jc * P:(jc + 1) * P]
                        )
                        nc.tensor.matmul(
                            E[:, jc, ic * P:(ic + 1) * P],
                            stg[:, :],
                            ident32,
                            is_transpose=True,
                            skip_group_check=True,
                        )

                # softmax: tS = exp(mn - E) in f16, Z row-sum fused (f32)
                for ic in range(CO):
                    nc.vector.tensor_reduce(
                        mn[:, ic:ic + 1], E[:, ic, :], AX.X, ALU.min
                    )
                for ic in range(CO):
                    nc.scalar.activation(
                        tS[:, ic, :], E[:, ic, :], AF.Exp,
                        bias=mn[:, ic:ic + 1], scale=-1.0,
                        accum_out=zs[:, ic:ic + 1],
                    )
                nc.vector.reciprocal(rg[:, :], zs[:, :])
                nc.vector.tensor_scalar_mul(rg[:, :], rg[:, :], g_col[:, :1])

                # tT[j, jc, i] via XBAR transposes of tS rows. On scalar:
                # its dep (exp) is also the last scalar op, so no false
                # ordering; sync must stay free for out-DMAs, which would
                # otherwise defer behind these and stall ot recycling.
                for ic in range(CO):
                    nc.scalar.dma_start_transpose(
                        tT[:, :, ic * P:(ic + 1) * P], tS[:, ic, :]
                    )
                st[b]["tT"] = tT
                st[b]["rg"] = rg

            def emit_mm2(b, prefetch_b):
                """mm2 + evac + out-DMA per n-window; interleaves the
                in-DMAs of batch `prefetch_b` (X16 recycles per window)."""
                X16w, tT, rg = st[b]["X16w"], st[b]["tT"], st[b]["rg"]
                y_b = y_out[b].rearrange("(co p) n -> p co n", p=P)
                if prefetch_b is not None:
                    make_x16(prefetch_b)
                for w in range(NW):
                    ot = otp.tile([P, CO, 512], F32, tag="ot")
                    for ic in range(CO):
                        ps2 = psum_acc.tile([P, 512], F32, tag="acc")
                        for jc in range(CO):
                            nc.tensor.matmul(
                                ps2[:, :],
                                tT[:, jc, ic * P:(ic + 1) * P],
                                X16w[w][:, jc, :],
                                start=(jc == 0), stop=(jc == CO - 1),
                            )
                        nc.vector.scalar_tensor_tensor(
                            ot[:, ic, :], ps2[:, :], rg[:, ic:ic + 1],
                            X16w[w][:, ic, :],
                            op0=ALU.mult, op1=ALU.add,
                        )
                    # out on sync HWDGE: SWDGE descriptors cost ~124ns/2KB
                    # vs HWDGE 17ns and would back up the DMA rings
                    nc.sync.dma_start(
                        y_b[:, :, w * 512:(w + 1) * 512], ot[:, :, :]
                    )
                    if prefetch_b is not None:
                        in_dma(prefetch_b, w)
                del st[b]

            loop_ctx = (
                tc.For_i(0, reps, 1) if reps > 1 else contextlib.nullcontext()
            )
            with loop_ctx:
                for k in range(Bs + 1):
                    if k == 0:
                        make_x16(0)
                        for w in range(NW):
                            in_dma(0, w)
                        if Bs > 1:
                            make_x16(1)
                            for w in range(NW):
                                in_dma(1, w)
                    if k < Bs:
                        emit_mm1(k)
                    if k >= 1:
                        nb = k + 1 if k + 1 < Bs else None
                        emit_mm2(k - 1, nb)

            if o["timing_io"]:
                ysb = stats.tile([1, 1], F32, tag="ysb")
                nc.sync.dma_start(
                    ysb[:1, :1], y_out[Bs - 1, C - 1:C, N - 1:N]
                )
                nc.sync.dma_start(yy_out[:1, :1], ysb[:1, :1])

    nc.compile()
    return nc


def get_nc(Bs=4, C=512, N=4096, reps=1, **opts):
    key = (Bs, C, N, reps, tuple(sorted(opts.items())))
    if key not in _CACHE:
        _CACHE[key] = _build(Bs, C, N, reps, **opts)
    return _CACHE[key]


def kernel(x, gamma):
    """Full inputs in, full output out. x [32, 512, 4096] f32, gamma [1] f32."""
    from concourse.bass_utils import run_bass_kernel_spmd

    x = np.ascontiguousarray(np.asarray(x, dtype=np.float32))
    gamma = np.ascontiguousarray(np.asarray(gamma, dtype=np.float32))
    B, C, N = x.shape
    n_cores = 8
    assert B % n_cores == 0
    Bs = B // n_cores

    nc = get_nc(Bs, C, N)
    in_maps = [
        {"x": x[i * Bs:(i + 1) * Bs], "gamma": gamma} for i in range(n_cores)
    ]
    res = run_bass_kernel_spmd(nc, in_maps, core_ids=list(range(n_cores)))
    return np.concatenate([r["y"] for r in res.results], axis=0)
